# revision 14
# baseline (speedup 1.0000x reference)
"""MeshConv (gnn_message_passing) Trainium2 kernel.

Math (per batch b):
    idx[e] = [e, ne0[e], ne1[e], ne2[e], ne3[e]]   (self + 4 neighbor edges)
    taps:  e0 = x[:, e],  ek = x[:, ne_{k-1}[e]]
    G = [e0, e1+e3, e2+e4, |e1-e3|, |e2-e4|]       (5 "taps" of 128 channels)
    out[o, e] = sum_{c,k} G[c, e, k] * W[o, c, k] + bias[o]

Strategy (8 NeuronCores):
  - Data parallel over (batch, edge-half): core = b*2 + h handles 15000 edges
    of batch b. Conv weight replicated.
  - Neighbor taps: SWDGE dma_gather in NON-transpose mode, with the 4 taps
    spread over 4 SWDGE queues. Queue q's descriptor generation runs on Q7
    core pair q (ucode: cpu_id / 2 == queue_num), so 4 queues generate in
    parallel -- measured 3.1x faster than the single-queue generation that
    paced the previous kernel (~8 ns/idx, 490 us busy of a 520 us kernel).
    Transpose-mode gathers are NOT multi-queue-safe (concurrent xbar sprays
    corrupt each other -- measured 75% mismatch); non-transpose gathers
    write plain 256 B rows and verified bit-exact across queues.
  - Each gathered tap tile [128 edge-part, sz/128, 128 ch] is flipped into
    matmul rhs layout [128 ch, sz edges] by ONE blockwise dma_start_transpose
    (HWDGE xbar, serialized on the SP/sync queue to keep xbar use exclusive).
  - Tap combines (add / sub) on DVE in bf16, |.| on ACT.
  - 5 accumulating bf16 matmuls per 128-output-half into fp32 PSUM; bias is
    fused into the PSUM->SBUF copy on ACT (Identity activation with bias AP),
    which emits bf16 -- the DRAM roundtrip of the output is halved vs f32.
"""

import os
import sys

sys.path.insert(0, "/opt/trn_rl_repo")

from contextlib import ExitStack

import ml_dtypes
import numpy as np

import concourse.bacc as bacc
import concourse.bass as bass
import concourse.tile as tile
from concourse import mybir

BF16 = ml_dtypes.bfloat16

P = 128          # partitions / in-channels
B, C, E, KT = 4, 128, 30000, 5
CO = 256         # out-channels
NCORES = 8
EH = E // 2      # edges per core (half a batch)
TILE = 3072      # max edges per gather macro-tile (multiple of 128)
# Per-tile gather wall = sz * ~8ns (4 taps run concurrently on 4 queues).
# The small trailing tiles shorten the post-last-gather pipeline tail.
TSZ = (3072, 3072, 3072, 3072, 2048, 768)
NT = len(TSZ)
EPAD = sum(TSZ)  # 15104 >= EH
CH = 480         # matmul chunk: psum free dim (480 f32 <= one 2KB bank)
SZW = [sz // 16 for sz in TSZ]
IDXTOT = 4 * sum(SZW)

_LAST_RESULTS = None  # BassKernelResults of the most recent run (for test.py)
_PROGRAM = None


def build_program(nt: int = NT) -> bass.Bass:
    nc = bacc.Bacc("TRN2", num_swdge_queues=4)
    xt = nc.declare_dram_parameter("xt", [E, C], mybir.dt.bfloat16, isOutput=False)
    x0 = nc.declare_dram_parameter("x0", [C, EPAD], mybir.dt.bfloat16, isOutput=False)
    idx = nc.declare_dram_parameter("idx", [P, IDXTOT], mybir.dt.int16, isOutput=False)
    wt = nc.declare_dram_parameter("wt", [P, KT * CO], mybir.dt.bfloat16, isOutput=False)
    bias = nc.declare_dram_parameter("bias", [P, 2], mybir.dt.float32, isOutput=False)
    out = nc.declare_dram_parameter("out", [CO, EH], mybir.dt.bfloat16, isOutput=True)

    with tile.TileContext(nc) as tc, ExitStack() as ctx:
        consts = ctx.enter_context(tc.tile_pool(name="consts", bufs=1))
        gpool = ctx.enter_context(tc.tile_pool(name="gath", bufs=2))
        tpool = ctx.enter_context(tc.tile_pool(name="texp", bufs=2))
        cpool = ctx.enter_context(tc.tile_pool(name="comb", bufs=2))
        opool = ctx.enter_context(tc.tile_pool(name="outs", bufs=2))
        psum = ctx.enter_context(tc.tile_pool(name="psum", bufs=3, space="PSUM"))

        # Idx prefetch split in two: tile 0's segment lands first so the
        # first gathers are not gated on the full index transfer.
        idx0_t = consts.tile([P, 4 * SZW[0]], mybir.dt.int16, tag="idx0")
        nc.sync.dma_start(out=idx0_t[:], in_=idx[:, : 4 * SZW[0]])
        idxr_t = consts.tile([P, IDXTOT - 4 * SZW[0]], mybir.dt.int16, tag="idxr")
        nc.sync.dma_start(out=idxr_t[:], in_=idx[:, 4 * SZW[0] :])
        wt_t = consts.tile([P, KT * CO], mybir.dt.bfloat16)
        nc.scalar.dma_start(out=wt_t[:], in_=wt[:])
        bias_t = consts.tile([P, 2], mybir.dt.float32)
        nc.scalar.dma_start(out=bias_t[:], in_=bias[:])

        toff = [sum(TSZ[:i]) for i in range(NT + 1)]
        ioff = [4 * sum(SZW[:i]) for i in range(NT + 1)]
        for t in range(nt):
            sz = TSZ[t]
            szw = sz // 16
            idx_t = idx0_t if t == 0 else idxr_t
            ib = 0 if t == 0 else ioff[t] - 4 * SZW[0]
            x0_t = gpool.tile([P, TILE], mybir.dt.bfloat16, tag="x0")
            nc.scalar.dma_start(
                out=x0_t[:, :sz], in_=x0[:, toff[t] : toff[t] + sz]
            )

            # Non-transpose gathers, tap k on SWDGE queue k: all 4 generate
            # concurrently. Output layout: edge j at [part j%128, row j//128],
            # 128 channels contiguous along free.
            g = [None] * 4
            for k in range(4):
                gk = gpool.tile([P, TILE], mybir.dt.bfloat16, tag=f"g{k}")
                nc.gpsimd.dma_gather(
                    gk[:, :sz].rearrange("p (r c) -> p r c", c=C),
                    xt[:],
                    idx_t[:, ib + k * szw : ib + (k + 1) * szw],
                    num_idxs=sz,
                    num_idxs_reg=sz,
                    elem_size=C,
                    transpose=False,
                    single_packet=False,
                    queue_num=k,
                )
                g[k] = gk

            # Blockwise xbar transpose into matmul rhs layout [128 ch, sz]:
            # out[c, r, p] = in[p, r*128 + c]  =>  tk[c, j] = x[ids[j], c].
            tt = [None] * 4
            for k in range(4):
                tk = tpool.tile([P, TILE], mybir.dt.bfloat16, tag=f"t{k}")
                nc.sync.dma_start_transpose(
                    out=tk[:, :sz].rearrange("p (r c) -> p r c", c=C),
                    in_=g[k][:, :sz],
                )
                tt[k] = tk

            pt = cpool.tile([P, TILE], mybir.dt.bfloat16, tag="p")
            nc.vector.tensor_tensor(
                out=pt[:, :sz], in0=tt[0][:, :sz], in1=tt[2][:, :sz], op=mybir.AluOpType.add
            )
            d13 = cpool.tile([P, TILE], mybir.dt.bfloat16, tag="d13")
            nc.vector.tensor_tensor(
                out=d13[:, :sz], in0=tt[0][:, :sz], in1=tt[2][:, :sz], op=mybir.AluOpType.subtract
            )
            qt = cpool.tile([P, TILE], mybir.dt.bfloat16, tag="q")
            nc.vector.tensor_tensor(
                out=qt[:, :sz], in0=tt[1][:, :sz], in1=tt[3][:, :sz], op=mybir.AluOpType.add
            )
            d24 = cpool.tile([P, TILE], mybir.dt.bfloat16, tag="d24")
            nc.vector.tensor_tensor(
                out=d24[:, :sz], in0=tt[1][:, :sz], in1=tt[3][:, :sz], op=mybir.AluOpType.subtract
            )
            # |.| in place on ACT: d13/d24 become the abs taps directly.
            nc.scalar.activation(
                out=d13[:, :sz], in_=d13[:, :sz], func=mybir.ActivationFunctionType.Abs
            )
            nc.scalar.activation(
                out=d24[:, :sz], in_=d24[:, :sz], func=mybir.ActivationFunctionType.Abs
            )

            # Batched output: evict psum chunks into one [128, sz] tile per
            # half, then ONE out-DMA per (tile, half). Keeping HWDGE DMAs to
            # 7 per tile (1 x0 + 4 transposes + 2 outs) matters: Tile
            # rotates HWDGE completions over 8 global DMAHW semaphore lanes,
            # and with ~18 DMAs/tile the lane aliasing chained gather waits
            # behind the previous tile's transposes/outs (measured: first
            # ~7 gathers fully serial at ~35 us cadence).
            taps = [(0, x0_t), (1, pt), (2, qt), (3, d13), (4, d24)]
            ob = [
                opool.tile([P, TILE], mybir.dt.bfloat16, tag=f"o{h}", name=f"ob{h}")
                for h in range(2)
            ]
            nch = (sz + CH - 1) // CH
            for ci in range(nch):
                w = min(CH, sz - ci * CH)
                for h in range(2):
                    ps = psum.tile([P, CH], mybir.dt.float32, tag=f"ps{h}")
                    for j, (k, rt) in enumerate(taps):
                        nc.tensor.matmul(
                            out=ps[:, :w],
                            lhsT=wt_t[:, k * CO + h * P : k * CO + h * P + P],
                            rhs=rt[:, ci * CH : ci * CH + w],
                            start=(j == 0),
                            stop=(j == len(taps) - 1),
                        )
                    nc.scalar.activation(
                        out=ob[h][:, ci * CH : ci * CH + w],
                        in_=ps[:, :w],
                        func=mybir.ActivationFunctionType.Identity,
                        bias=bias_t[:, h : h + 1],
                    )
            ncols = min(sz, EH - toff[t])
            for h in range(2):
                nc.sync.dma_start(
                    out=out[h * P : (h + 1) * P, toff[t] : toff[t] + ncols],
                    in_=ob[h][:, :ncols],
                )
    nc.finalize()
    return nc


def make_in_maps(x, ne_idx, conv_w, conv_b):
    xs = np.asarray(x)[..., 0]  # [B, C, E] f32
    xtb = np.ascontiguousarray(xs.transpose(0, 2, 1)).astype(BF16)  # [B, E, C]
    x0b = xs.astype(BF16)  # [B, C, E]

    wt_host = np.zeros((P, KT * CO), np.float32)
    for k in range(KT):
        wt_host[:, k * CO : (k + 1) * CO] = conv_w[:, :, 0, k].T
    wt_host = wt_host.astype(BF16)
    bias_host = np.ascontiguousarray(np.asarray(conv_b).reshape(2, P).T).astype(
        np.float32
    )

    ne = np.asarray(ne_idx)
    in_maps = []
    for core in range(NCORES):
        b, h = divmod(core, 2)
        lo = h * EH
        x0c = np.zeros((C, EPAD), BF16)
        x0c[:, :EH] = x0b[b][:, lo : lo + EH]
        idxc = np.zeros((EH, 4), np.int16)
        idxc[:] = ne[b, lo : lo + EH, :].astype(np.int16)
        rep = np.zeros((P, IDXTOT), np.int16)
        woff = 0
        eoff = 0
        for t, sz in enumerate(TSZ):
            szw = sz // 16
            rsz = min(sz, EH - eoff)
            for k in range(4):
                ids = np.zeros(sz, np.int16)
                ids[:rsz] = idxc[eoff : eoff + rsz, k]
                blk = ids.reshape(szw, 16).T  # [16, szw]
                rep[:, woff : woff + szw] = np.tile(blk, (8, 1))
                woff += szw
            eoff += sz
        in_maps.append(
            {
                "xt": xtb[b],
                "x0": x0c,
                "idx": rep,
                "wt": wt_host,
                "bias": bias_host,
            }
        )
    return in_maps


def kernel(x, ne_idx, conv_w, conv_b):
    global _LAST_RESULTS, _PROGRAM
    from concourse.bass_utils import run_bass_kernel_spmd

    in_maps = make_in_maps(x, ne_idx, conv_w, conv_b)
    if _PROGRAM is None:
        _PROGRAM = build_program()
    res = run_bass_kernel_spmd(
        _PROGRAM,
        in_maps,
        core_ids=list(range(NCORES)),
        trace=bool(os.environ.get("KERNEL_TRACE")),
    )
    _LAST_RESULTS = res

    out_full = np.zeros((B, CO, E), np.float32)
    for core in range(NCORES):
        b, h = divmod(core, 2)
        out_full[b, :, h * EH : (h + 1) * EH] = res.results[core]["out"].astype(
            np.float32
        )
    return out_full[..., None]
